# revision 32
# baseline (speedup 1.0000x reference)
"""Trainium2 Bass kernel for nn_MultiHeadAttention_60559038873660.

Reference math (faithful to the source bug: attention is contracted with the
projected K, not V, so v/Wv are dead inputs):
    qp = q @ Wq.T ; kp = k @ Wk.T
    head split via reshape(b, l, 64, 16): head n takes strided columns {d*16+n}
    S = Qh @ Kh.T / 8 ; A = softmax(S, axis=m) ; X = A @ Kh ; out = X @ Wo.T

v2 design (from trace analysis of the 275us baseline):
  - fp16 everywhere (same PE rate as bf16, 8x better accuracy; frees error
    budget for the DVE fast-exp below).
  - 8 cores = 2 batches x 4 head-groups (4 heads each), host sums partials.
  - S matmuls are row-tiled (heads at partitions 0:64 / 64:128 run
    concurrently on disjoint PE row-groups -- auto tile_position).
  - exp(S/8) split across engines: even m-chunks on ScalarE (ACT table exp),
    odd m-chunks on VectorE via a one-instruction Schraudolph fp16-bits exp:
    bits = round(S*(1024*log2e/8) + 15360 - C + 0.5) stored as int16 and
    bitcast to f16.  CPU-sim rel err at this 8/16 split: 4.3e-3 (gate 2e-2).
  - q/k staged whole in SBUF upfront on two DMA queues so projection fillers
    never stall the PE queue; projections/transposes/out-proj are woven into
    the attention loop as fillers to keep PE dense (HAM stays at K=8/8).
  - deferred normalization: X psum is copied out fast (raw + den row), the
    reciprocal/broadcast/multiply chain trails off-critical-path; the final
    multiply runs on GpSimd (SBUF-only op, GPSIMD has no PSUM port).
  - out-projection tiles are emitted as g1 fillers; po psum -> f16 tile on
    ScalarE -> DMA; output DRAM tensor is f16, host accumulates in fp32.
"""

import contextlib
import ctypes
import os
import sys
import types

import numpy as np

import concourse.bacc as bacc
import concourse.tile as tile
from concourse import mybir
from concourse.bass import ds, ts
from concourse.bass_utils import run_bass_kernel_spmd


def _install_ntff_hook():
    """Provide antenv.axon_hooks if the image lacks it, wiring NTFF
    profiling straight into libaxon_pjrt.so (same ABI trn_boot uses)."""
    try:
        import antenv.axon_hooks  # noqa: F401
        return
    except ImportError:
        pass
    mod = types.ModuleType("antenv.axon_hooks")
    holder = [None]
    mod.set_axon_ntff_profile_hook = lambda h: holder.__setitem__(0, h)
    mod.get_axon_ntff_profile_hook = lambda: holder[0]
    sys.modules["antenv.axon_hooks"] = mod
    try:
        import antenv
        antenv.axon_hooks = mod
    except ImportError:
        pass

    so_path = "/opt/axon/libaxon_pjrt.so"
    if not os.path.exists(so_path):
        return
    lib = ctypes.CDLL(so_path)
    if not hasattr(lib, "axon_start_nrt_profile"):
        return
    lib.axon_start_nrt_profile.argtypes = [ctypes.POINTER(ctypes.c_int64), ctypes.c_size_t]
    lib.axon_start_nrt_profile.restype = ctypes.c_int64
    lib.axon_stop_nrt_profile.argtypes = [ctypes.c_char_p]
    lib.axon_stop_nrt_profile.restype = ctypes.c_int64

    @contextlib.contextmanager
    def _hook(output_dir, device_ids):
        import jax
        jax.devices()
        if device_ids:
            ids = (ctypes.c_int64 * len(device_ids))(*device_ids)
            rc = lib.axon_start_nrt_profile(ids, len(device_ids))
        else:
            rc = lib.axon_start_nrt_profile(None, 0)
        if rc != 0:
            raise RuntimeError(f"axon_start_nrt_profile rc={rc}")
        try:
            yield
        finally:
            n = lib.axon_stop_nrt_profile(str(output_dir).encode())
            print(f"profile: {n} file(s) written to {output_dir}", file=sys.stderr)

    mod.set_axon_ntff_profile_hook(_hook)


_install_ntff_hook()

f32 = mybir.dt.float32
f16 = mybir.dt.float16
i16 = mybir.dt.int16
Exp = mybir.ActivationFunctionType.Exp
MUL = mybir.AluOpType.mult
ADD = mybir.AluOpType.add

P = 128
DIM = 1024
NH = 16
HD = 64
HPC = 4          # heads per core
CW = HPC * HD    # 256 channel columns per core
G = CW // P      # 2 channel groups of 128 (2 heads each)
KC = DIM // P    # 8 contraction chunks for projections

# Schraudolph fp16-bits exp: es = exp(S/8) ~= bitcast_f16(int16(S*A + B))
SCH_A = 1024.0 * 1.4426950408889634 / 8.0        # 184.6649...
SCH_C = 38.0
SCH_B = 15360.0 - SCH_C + 0.5                    # +0.5: robust to trunc cast
DVE_EXP = bool(int(os.environ.get("MHA_DVE_EXP", "1")))
OPROJ_ACT_COPY = bool(int(os.environ.get("MHA_OPROJ_ACT", "1")))
DEBUG_DUMP = bool(int(os.environ.get("MHA_DEBUG", "0")))

_cache = {}


def _build(L, M):
    NT = 512
    LT = L // NT              # q tiles
    MT = M // NT              # k tiles
    MG = M // P               # attention m-chunks
    L5 = L // NT              # attention l-strips
    LC = L // P               # out-proj l-chunks
    JT = DIM // NT            # out-proj j tiles

    nc = bacc.Bacc()
    qT = nc.declare_dram_parameter("qT", [DIM, L], f16, isOutput=False)
    kT = nc.declare_dram_parameter("kT", [DIM, M], f16, isOutput=False)
    wqT = nc.declare_dram_parameter("wqT", [DIM, CW], f16, isOutput=False)
    wkT = nc.declare_dram_parameter("wkT", [DIM, CW], f16, isOutput=False)
    woT = nc.declare_dram_parameter("woT", [CW, DIM], f16, isOutput=False)
    out = nc.declare_dram_parameter("out", [L, DIM], f16, isOutput=True)
    if DEBUG_DUMP:
        den_dram = nc.declare_dram_parameter("den_scratch", [HPC, L], f32, isOutput=True)
        rden_dram = nc.declare_dram_parameter("rden_scratch", [HPC, L], f32, isOutput=True)
    else:
        den_dram = nc.dram_tensor("den_scratch", [HPC, L], f32)
        rden_dram = nc.dram_tensor("rden_scratch", [HPC, L], f32)
    dbg = {}
    if DEBUG_DUMP:
        dbg["qhT"] = nc.declare_dram_parameter("dbg_qhT", [P, G * L], f16, isOutput=True)
        dbg["khT"] = nc.declare_dram_parameter("dbg_khT", [P, G * M], f16, isOutput=True)
        dbg["khp"] = nc.declare_dram_parameter("dbg_khp", [P, MG * HPC * P], f16, isOutput=True)
        dbg["xu"] = nc.declare_dram_parameter("dbg_xu", [P, G * L], f16, isOutput=True)
        dbg["rdbc"] = nc.declare_dram_parameter("dbg_rdbc", [P, G * L], f32, isOutput=True)
        dbg["es0"] = nc.declare_dram_parameter("dbg_es0", [P, 2 * NT], f16, isOutput=True)
        dbg["es1"] = nc.declare_dram_parameter("dbg_es1", [P, 2 * NT], f16, isOutput=True)

    from concourse.masks import make_identity

    with tile.TileContext(nc) as tc:
        with (
            tc.tile_pool(name="singles", bufs=1) as singles,
            tc.tile_pool(name="esa", bufs=20) as esa_pool,
            tc.tile_pool(name="esb", bufs=20) as esb_pool,
            tc.tile_pool(name="dstp", bufs=2) as dstp,
            tc.tile_pool(name="dsp", bufs=2) as dsp_pool,
            tc.tile_pool(name="ot", bufs=3) as opool,
        ):
            # --- resident SBUF tensors -------------------------------------
            qsb = singles.tile([P, KC, L], f16)      # full q^T staged
            ksb = singles.tile([P, KC, M], f16)      # full k^T staged
            wq_sb = singles.tile([P, KC, CW], f16)
            wk_sb = singles.tile([P, KC, CW], f16)
            wo_sb = singles.tile([P, G, DIM], f16)
            qhT = singles.tile([P, G, L], f16)
            khT = singles.tile([P, G, M], f16)
            khp = singles.tile([P, MG, HPC, P], f16)  # [m, head, chan+ones+pad]
            xu = singles.tile([P, G, L], f16)
            rdbc = singles.tile([P, G, L], f32)
            ident = singles.tile([P, P], f16)
            ones_col = singles.tile([P, 1], f32)
            trash_col = singles.tile([P, 1], f32)

            # input staging split across all three DMA-capable queues so
            # k-proj tile t's inputs land by ~2*t us
            def _kslice(t):
                return kT[:, ts(t, NT)].rearrange("(kc p) m -> p kc m", p=P)
            nc.scalar.dma_start(wk_sb, wkT.rearrange("(kc p) c -> p kc c", p=P))
            nc.sync.dma_start(ksb[:, :, ts(0, NT)], _kslice(0))
            nc.scalar.dma_start(ksb[:, :, ts(2, NT)], _kslice(2))
            nc.sync.dma_start(ksb[:, :, ts(1, NT)], _kslice(1))
            nc.scalar.dma_start(ksb[:, :, ts(3, NT)], _kslice(3))
            nc.sync.dma_start(wq_sb, wqT.rearrange("(kc p) c -> p kc c", p=P))
            nc.sync.dma_start(wo_sb, woT.rearrange("(g p) j -> p g j", p=P))
            for t in range(LT):
                nc.gpsimd.dma_start(
                    qsb[:, :, ts(t, NT)],
                    qT[:, ts(t, NT)].rearrange("(kc p) l -> p kc l", p=P))

            make_identity(nc, ident)
            nc.gpsimd.memset(khp, 0.0)
            nc.gpsimd.memset(ones_col, 1.0)
            nc.gpsimd.memset(khp[:, :, :, HD:HD + 1], 1.0)

            with (
                tc.tile_pool(name="psSA", bufs=2, space="PSUM") as psSA,
                tc.tile_pool(name="psSB", bufs=2, space="PSUM") as psSB,
                tc.tile_pool(name="psX", bufs=2, space="PSUM") as psX,
                tc.tile_pool(name="psW", bufs=2, space="PSUM") as psW,
            ):
                def proj(src_sb, w_sb, dst, tt, g):
                    ps = psW.tile([P, NT], f32, tag="pw")
                    for kc in range(KC):
                        nc.tensor.matmul(ps, lhsT=w_sb[:, kc, ts(g, P)],
                                         rhs=src_sb[:, kc, ts(tt, NT)],
                                         start=(kc == 0), stop=(kc == KC - 1))
                    nc.vector.tensor_copy(dst[:, g, ts(tt, NT)], ps)

                def ktrans(g, mc):
                    # carve an f16 [P, P] transpose target out of a "pw"
                    # f32 bank via bitcast (no spare PSUM bank for a 4th tag)
                    ps = psW.tile([P, NT], f32, tag="pw")
                    tr = ps[:, :].bitcast(f16)[:, 0:P]
                    nc.tensor.transpose(tr, khT[:, g, ts(mc, P)], ident)
                    for hh in range(2):
                        nc.scalar.copy(out=khp[:, mc, 2 * g + hh, 0:HD],
                                       in_=tr[:, ts(hh, HD)])

                def oproj(lc, jt):
                    po = psW.tile([P, NT], f32, tag="pw")
                    nc.tensor.matmul(po, lhsT=xu[:, 0, ts(lc, P)],
                                     rhs=wo_sb[:, 0, ts(jt, NT)],
                                     start=True, stop=False)
                    nc.tensor.matmul(po, lhsT=xu[:, 1, ts(lc, P)],
                                     rhs=wo_sb[:, 1, ts(jt, NT)],
                                     start=False, stop=True)
                    ot = opool.tile([P, NT], f16, tag="ot")
                    if OPROJ_ACT_COPY:
                        nc.scalar.copy(out=ot, in_=po)
                    else:
                        nc.vector.tensor_copy(ot, po)
                    nc.sync.dma_start(out[ts(lc, P), ts(jt, NT)], ot)

                # deferred units: emission must precede the consumer's
                # emission (Tile deps follow program order), so strip0 emits
                # its own kproj/ktrans just-in-time and later strips pull
                # prep units for the next group / out-proj via callbacks.
                units = []

                def pop_unit():
                    if units:
                        units.pop(0)()

                # --- head: minimum work before attention g0 ----------------
                proj(ksb, wk_sb, khT, 0, 0)
                ktrans(0, 0)
                ktrans(0, 1)
                proj(qsb, wq_sb, qhT, 0, 0)

                # --- attention -------------------------------------------
                # X for strip n is emitted during strip n+1's loop so the PE
                # always has exp-independent matmuls to chew while the psS
                # ring waits on ACT/DVE -- keeps HAM at K=8/8.  The es tiles
                # of a strip stay live one extra strip (es pool sized for it).
                tail_recip = []
                tail_mul = []

                class XJob:
                    def __init__(self, g, l5, es_list):
                        self.g, self.l5, self.es = g, l5, es_list
                        self.xpsA = None
                        self.xpsB = None

                    def step(self, mc):
                        if mc == 0:
                            self.xpsA = psX.tile([P, NT], f32, tag="x")
                            self.xpsB = psX.tile([P, NT], f32, tag="x")
                        hA, hB = 2 * self.g, 2 * self.g + 1
                        ea, eb = self.es[mc]
                        nc.tensor.matmul(self.xpsA, lhsT=khp[:, mc, hA, :],
                                         rhs=ea,
                                         start=(mc == 0), stop=(mc == MG - 1))
                        nc.tensor.matmul(self.xpsB, lhsT=khp[:, mc, hB, :],
                                         rhs=eb[:, :].bitcast(f16) if DVE_EXP
                                         else eb,
                                         start=(mc == 0), stop=(mc == MG - 1))
                        if mc == MG - 1:
                            self.finish()

                    def finish(self):
                        g, l5 = self.g, self.l5
                        lsl = ts(l5, NT)
                        for hh, xps in ((0, self.xpsA), (1, self.xpsB)):
                            h = 2 * g + hh
                            pb = hh * HD
                            nc.vector.tensor_copy(xu[pb:pb + HD, g, lsl],
                                                  xps[0:HD])
                            dstg = dstp.tile([1, NT], f32, tag="dst")
                            nc.vector.tensor_copy(dstg, xps[HD:HD + 1])
                            nc.gpsimd.dma_start(den_dram[h:h + 1, lsl], dstg)
                            dsp_t = dsp_pool.tile([P, NT // P], f32, tag="dsp")
                            nc.gpsimd.dma_start(
                                dsp_t,
                                den_dram[h, lsl].rearrange("(p f) -> p f", p=P))

                            def _recip(h=h, hh=hh, dsp_t=dsp_t, lsl=lsl, g=g):
                                rden_t = dsp_pool.tile([P, NT // P], f32,
                                                       tag="rdn")
                                nc.vector.reciprocal(rden_t, dsp_t)
                                nc.gpsimd.dma_start(
                                    rden_dram[h, lsl].rearrange(
                                        "(p f) -> p f", p=P), rden_t)
                                nc.gpsimd.dma_start(
                                    rdbc[ts(hh, HD), g, lsl],
                                    rden_dram[h:h + 1, lsl].to_broadcast(
                                        [HD, NT]))

                            def _mul(hh=hh, pb=pb, lsl=lsl, g=g, l5=l5):
                                nc.vector.tensor_mul(
                                    xu[pb:pb + HD, g, lsl],
                                    xu[pb:pb + HD, g, lsl],
                                    rdbc[ts(hh, HD), g, lsl])
                                if g == 1 and hh == 1:
                                    # xu for this l-range complete: queue its
                                    # out-projection tiles
                                    for lc in range(l5 * (NT // P),
                                                    (l5 + 1) * (NT // P)):
                                        for jt in range(JT):
                                            units.append(
                                                lambda lc=lc, jt=jt:
                                                oproj(lc, jt))

                            tail_recip.append(_recip)
                            tail_mul.append(_mul)

                def attn_strip(g, l5, pre_mc, pop_every, pop_from, xjob):
                    lsl = ts(l5, NT)

                    def emit_sp(mc):
                        if pre_mc is not None:
                            pre_mc(mc)
                        spsA = psSA.tile([P, NT], f32, tag="sa")
                        spsB = psSB.tile([P, NT], f32, tag="sb")
                        nc.tensor.matmul(spsA, lhsT=khT[0:HD, g, ts(mc, P)],
                                         rhs=qhT[0:HD, g, lsl],
                                         start=True, stop=True)
                        nc.tensor.matmul(spsB, lhsT=khT[HD:P, g, ts(mc, P)],
                                         rhs=qhT[HD:P, g, lsl],
                                         start=True, stop=True)
                        return (spsA, spsB)

                    es_list = []
                    sq = [emit_sp(0), emit_sp(1)]
                    for mc in range(MG):
                        if mc == 3:
                            while tail_recip:
                                tail_recip.pop(0)()
                        if mc == 6:
                            while tail_mul:
                                tail_mul.pop(0)()
                        if mc + 2 < MG:
                            sq.append(emit_sp(mc + 2))
                        ea = esa_pool.tile([P, NT], f16, tag="ea")
                        eb = esb_pool.tile([P, NT], i16 if DVE_EXP else f16,
                                           tag="eb")
                        es_list.append((ea, eb))
                        spsA, spsB = sq.pop(0)
                        nc.scalar.activation(ea, spsA, Exp, scale=0.125)
                        if DVE_EXP:
                            nc.vector.tensor_scalar(
                                eb, spsB, SCH_A, SCH_B, MUL, ADD)
                        else:
                            nc.scalar.activation(eb, spsB, Exp,
                                                 scale=0.125)
                        if DEBUG_DUMP and g == 0 and l5 == 0 and mc < 2:
                            dtgt = dbg["es0" if mc == 0 else "es1"]
                            nc.gpsimd.dma_start(dtgt[:, 0:NT], ea)
                            nc.gpsimd.dma_start(
                                dtgt[:, NT:2 * NT],
                                eb[:, :].bitcast(f16) if DVE_EXP else eb)
                        if xjob is not None:
                            xjob.step(mc)
                        if pop_every and mc >= pop_from and (
                                mc % pop_every == pop_every - 1):
                            pop_unit()
                    return XJob(g, l5, es_list)

                # strip0 JIT-emits its own k tiles + transposes 2 mc ahead
                kdone = [True, False, False, False]

                def pre0(mc):
                    t = min(3, (mc + 2) // 4)
                    if not kdone[t]:
                        proj(ksb, wk_sb, khT, t, 0)
                        kdone[t] = True
                    if mc >= 2:
                        ktrans(0, mc)

                xjob = attn_strip(0, 0, pre0, 0, 0, None)

                # g1 prep spread over g0 strips 1..3 (24 units / 24 slots)
                for t in range(MT):
                    units.append(lambda t=t: proj(ksb, wk_sb, khT, t, 1))
                    for j in range(4):
                        units.append(lambda t=t, j=j: ktrans(1, 4 * t + j))
                for t in range(LT):
                    units.append(lambda t=t: proj(qsb, wq_sb, qhT, t, 1))

                for l5 in range(1, L5):
                    proj(qsb, wq_sb, qhT, l5, 0)
                    xjob = attn_strip(0, l5, None, 2, 0, xjob)
                while units:
                    pop_unit()

                for l5 in range(L5):
                    xjob = attn_strip(1, l5, None, 1, 7, xjob)

                # tail: bare X loop for the last strip
                for mc in range(MG):
                    if mc == 3:
                        while tail_recip:
                            tail_recip.pop(0)()
                    if mc == 6:
                        while tail_mul:
                            tail_mul.pop(0)()
                    xjob.step(mc)
                    if mc >= 7:
                        pop_unit()
                while tail_recip:
                    tail_recip.pop(0)()
                while tail_mul:
                    tail_mul.pop(0)()
                while units:
                    pop_unit()

                if DEBUG_DUMP:
                    nc.gpsimd.dma_start(dbg["qhT"][:, :], qhT.rearrange("p g l -> p (g l)"))
                    nc.gpsimd.dma_start(dbg["khT"][:, :], khT.rearrange("p g m -> p (g m)"))
                    nc.gpsimd.dma_start(dbg["khp"][:, :], khp.rearrange("p a b c -> p (a b c)"))
                    nc.gpsimd.dma_start(dbg["xu"][:, :], xu.rearrange("p g l -> p (g l)"))
                    nc.gpsimd.dma_start(dbg["rdbc"][:, :], rdbc.rearrange("p g l -> p (g l)"))

    nc.finalize()
    return nc


def _get_nc(L, M):
    key = (L, M, DVE_EXP, OPROJ_ACT_COPY, DEBUG_DUMP)
    if key not in _cache:
        _cache[key] = _build(L, M)
    return _cache[key]


# head-major channel permutation: new channel c = h*64+d <- original column d*16+h
_PERM = np.array([(c % HD) * NH + c // HD for c in range(DIM)])

last_exec_time_ns = None
last_results = None


def kernel(q, k, v, Wq, Wk, Wv, Wo):  # noqa: ARG001 - v/Wv dead in reference
    global last_exec_time_ns, last_results
    q = np.asarray(q, np.float32)
    k = np.asarray(k, np.float32)
    Wq = np.asarray(Wq, np.float32)
    Wk = np.asarray(Wk, np.float32)
    Wo = np.asarray(Wo, np.float32)
    B, L, _ = q.shape
    M = k.shape[1]

    Wq_p = Wq[_PERM]            # (1024, 1024) head-major rows
    Wk_p = Wk[_PERM]
    WoT_p = Wo[:, _PERM].T      # (1024 c, 1024 j)

    qT = [np.ascontiguousarray(q[b].T).astype(np.float16) for b in range(B)]
    kT = [np.ascontiguousarray(k[b].T).astype(np.float16) for b in range(B)]
    wqT = [np.ascontiguousarray(Wq_p[hg * CW:(hg + 1) * CW, :].T).astype(np.float16)
           for hg in range(4)]
    wkT = [np.ascontiguousarray(Wk_p[hg * CW:(hg + 1) * CW, :].T).astype(np.float16)
           for hg in range(4)]
    woT = [np.ascontiguousarray(WoT_p[hg * CW:(hg + 1) * CW, :]).astype(np.float16)
           for hg in range(4)]

    in_maps = []
    for core in range(8):
        b, hg = divmod(core, 4)
        in_maps.append({"qT": qT[b], "kT": kT[b], "wqT": wqT[hg],
                        "wkT": wkT[hg], "woT": woT[hg]})

    nc = _get_nc(L, M)
    trace = bool(int(os.environ.get("MHA_TRACE", "0")))
    res = run_bass_kernel_spmd(nc, in_maps, core_ids=list(range(8)), trace=trace)
    last_results = res
    last_exec_time_ns = res.exec_time_ns

    out = np.zeros((B, L, DIM), np.float32)
    for core in range(8):
        b = core // 4
        out[b] += np.asarray(res.results[core]["out"], np.float32)
    return out
